# revision 1
# baseline (speedup 1.0000x reference)
"""BiDAF-style attention kernel for Trainium2, data-parallel over batch on 8 cores.

Shapes (hardcoded): B=16, C=2048, Q=128, E=200, O=128.
Each core processes 2 batches. All matmuls in float32r (moving dim >= 256),
softmax without max-shift (scores ~ N(0,1), exp never overflows).

Layout: feature-on-partitions for everything contracted over e/f. Host ships
raw transposes of x_contexts / x_questions plus the natural layouts; the
projection computes out^T [o, c] and the host transposes back.

Softmax-factor cancellation: E/sum(E) is invariant to per-output-index
exponential offsets, so the q-softmax orientation folds only s_q (ACT exp
bias) and both orientations get s_c via lhsq = w3*xqT + w1; normalizers come
free (ones column in xcn for z, a ones-vector matmul for r, with 1/r
partition-broadcast as a K=1 matmul into PSUM).
"""

import numpy as np

import concourse.bass as bass
import concourse.mybir as mybir
from concourse import bacc
from concourse.bass import MemorySpace
from concourse.tile import TileContext
from concourse.bass_utils import run_bass_kernel_spmd

B, C, Q, E, O = 16, 2048, 128, 200, 128
NB = 2          # batches per core
NCORES = 8
EPAD = 256      # padded e/q for small-N matmuls (float32r needs N>=256)
CCH = 512       # c chunk size
NCH = C // CCH  # 4
CT = 128        # c tile (partitions)
NCT = C // CT   # 16
F32 = mybir.dt.float32
F32R = mybir.dt.float32r
EXP = mybir.ActivationFunctionType.Exp

_CACHE = {}


def _build(num_devices=NCORES, reps=1):
    nc = bacc.Bacc("TRN2", target_bir_lowering=False, debug=False,
                   num_devices=num_devices)

    d_xcta = nc.dram_tensor("xcta", [NB, 128, C], F32R, kind="ExternalInput")
    d_xctb = nc.dram_tensor("xctb", [NB, 73, C], F32R, kind="ExternalInput")
    # xcn_shuf[b, ct, p, :] = [x_contexts[b, ct*128+p, :], 1.0, 0-pad]
    d_xcn = nc.dram_tensor("xcn", [NB, NCT, 128, 204], F32R,
                           kind="ExternalInput")
    # xq_pack[b, p, :] = [wcols row p (6), xqT[0:128] row p,
    #                     xqT[128:200] row p (pad), xqn row p]
    d_xq = nc.dram_tensor("xq", [NB, 128, 462], F32R, kind="ExternalInput")
    # wp chunks padded to 128 rows each: [8, 128, O]
    d_wp = nc.dram_tensor("wp", [8, 128, O], F32R, kind="ExternalInput")
    d_out = nc.dram_tensor("out_t", [NB, O, C], F32, kind="ExternalOutput")

    # wp row chunks paired with featsT row chunks
    WP_CH = [(0, 128), (128, 201), (201, 329), (329, 401),
             (401, 529), (529, 601), (601, 729), (729, 801)]

    def mmr(ps, lhsT, rhs, start=True, stop=True):
        nc.tensor.matmul(ps, lhsT.bitcast(F32R), rhs.bitcast(F32R),
                         start=start, stop=stop)

    with TileContext(nc) as tc:
        with (
            tc.tile_pool(name="singles", bufs=1) as singles,
            tc.tile_pool(name="inputs", bufs=2) as inputs,
            tc.tile_pool(name="work", bufs=1) as work,
            tc.tile_pool(name="work2", bufs=2) as work2,
            tc.tile_pool(name="small", bufs=4) as small,
            tc.tile_pool(name="ps_big", bufs=3, space=MemorySpace.PSUM) as ps_big,
            tc.tile_pool(name="ps_t", bufs=1, space=MemorySpace.PSUM) as ps_t_pool,
            tc.tile_pool(name="ps_cq", bufs=2, space=MemorySpace.PSUM) as ps_cq,
            tc.tile_pool(name="ps_sc", bufs=2, space=MemorySpace.PSUM) as ps_sc,
                    ):
            # ---- batch-independent constants ----
            wp_all = singles.tile([128, 8, O], F32R, tag="wp")
            wp_sb = [wp_all[0:(r1 - r0), k, :]
                     for k, (r0, r1) in enumerate(WP_CH)]
            onecol = singles.tile([128, 1], F32R, tag="onecol")
            nc.vector.memset(onecol.bitcast(F32), 1.0)
            ones_row = singles.tile([1, 128], F32R, tag="ones_row")
            nc.vector.memset(ones_row.bitcast(F32), 1.0)

            for rep in range(reps):
              for b in range(NB):
                # ---- input DMAs (few + large; HWDGE costs 625ns each) ----
                xq = inputs.tile([128, 462], F32R, tag="xq")
                nc.sync.dma_start(out=xq, in_=d_xq.ap()[b])
                w1a = xq[:, 0:1]
                w1b = xq[0:72, 1:2]
                w2a = xq[:, 2:3]
                w2b = xq[0:72, 3:4]
                w3a = xq[:, 4:5]
                w3b = xq[0:72, 5:6]
                xqta = xq[:, 6:134]
                xqtb = xq[0:72, 134:262]
                xqn = xq[:, 262:462]
                xcta = inputs.tile([128, C], F32R, tag="xcta")
                xctb = inputs.tile([73, C], F32R, tag="xctb")
                for ch in range(NCH):
                    sl = slice(ch * CCH, (ch + 1) * CCH)
                    nc.sync.dma_start(out=xcta[:, sl],
                                      in_=d_xcta.ap()[b, :, sl])
                    nc.sync.dma_start(out=xctb[:, sl],
                                      in_=d_xctb.ap()[b, :, sl])
                xcn = inputs.tile([128, NCT, EPAD], F32R, tag="xcn")
                xcn_src = d_xcn.ap()[b].rearrange("n p m -> p n m")
                for g in range(4):
                    gs = slice(4 * g, 4 * g + 4)
                    nc.sync.dma_start(out=xcn[:, gs, 0:204],
                                      in_=xcn_src[:, gs, :])
                if rep == 0 and b == 0:
                    nc.sync.dma_start(out=wp_all,
                                      in_=d_wp.ap().rearrange("k p o -> p k o"))

                # ---- question-side lhsT prep (pure compute, no DMA) ----
                # lhsq[:, q] = w3*xqT[:, q] + w1  (the +w1 adds s_c to both
                # score orientations; it cancels in S1 = E/r where unwanted)
                lhsq_a = work.tile([128, EPAD], F32R, tag="lhsq_a")
                nc.vector.memset(lhsq_a.bitcast(F32), 0.0)
                nc.vector.scalar_tensor_tensor(
                    lhsq_a[:, 0:Q], xqta, w3a.bitcast(F32),
                    w1a.broadcast_to([128, Q]),
                    op0=mybir.AluOpType.mult, op1=mybir.AluOpType.add)
                lhsq_b = work.tile([72, EPAD], F32R, tag="lhsq_b")
                nc.vector.memset(lhsq_b.bitcast(F32), 0.0)
                nc.vector.scalar_tensor_tensor(
                    lhsq_b[:, 0:Q], xqtb, w3b.bitcast(F32),
                    w1b.broadcast_to([72, Q]),
                    op0=mybir.AluOpType.mult, op1=mybir.AluOpType.add)
                # s_q column [q, 1] = xqT^T @ w2 -> exp_qc bias.
                # (exp(s_c) factors cancel in S1 = E/r, exp(s_q) factors
                # cancel in S2 = E/z, so each orientation only needs its
                # per-contraction-index term.)
                ps_sqc = ps_sc.tile([Q, 4], F32, tag="ps_sc")
                mmr(ps_sqc[:, 0:2], xqta, xq[:, 2:4], start=True,
                    stop=False)
                mmr(ps_sqc[:, 0:2], xqtb, xq[0:72, 3:5], start=False,
                    stop=True)
                sq_col = small.tile([Q, 1], F32, tag="sq_col")
                nc.vector.tensor_copy(sq_col, ps_sqc[:, 0:1])

                # ---- scores^T [q, c]: exp + z accum; r row; S1^T, chunked --
                eqc = work2.tile([Q, C], F32R, tag="eqc")
                s1t = work2.tile([Q, C], F32R, tag="s1t")
                rrow = work.tile([1, C], F32R, tag="rrow")
                for ch in range(NCH):
                    sl = slice(ch * CCH, (ch + 1) * CCH)
                    ps = ps_big.tile([128, CCH], F32, tag="ps_big")
                    mmr(ps[0:Q, :], lhsq_a[:, 0:Q], xcta[:, sl],
                        start=True, stop=False)
                    mmr(ps[0:Q, :], lhsq_b[:, 0:Q], xctb[0:72, sl],
                        start=False, stop=True)
                    nc.scalar.activation(out=eqc[:, sl], in_=ps[0:Q, :],
                                         func=EXP, bias=sq_col)
                # ---- scores [c, q] per c-tile + exp -> E_cq; t accumulate --
                ecq = work.tile([128, NCT, Q], F32R, tag="ecq")
                # cq-MMs with the r-matmuls interleaved every 4th tile:
                # r depends only on eqc (done in the qc phase), so it is
                # ready work that fills the ACT-exp-paced PE bubbles and
                # starts the 1/r pipeline early.
                for ct in range(NCT):
                    tsl = slice(ct * CT, (ct + 1) * CT)
                    pool = ps_cq if ct % 2 == 0 else ps_t_pool
                    tagn = "ps_cq" if ct % 2 == 0 else "ps_t"
                    ps = pool.tile([128, EPAD], F32, tag=tagn)
                    mmr(ps, xcta[:, tsl], lhsq_a, start=True, stop=False)
                    mmr(ps, xctb[0:72, tsl], lhsq_b, start=False, stop=True)
                    nc.scalar.activation(out=ecq[:, ct, :], in_=ps[:, 0:Q],
                                         func=EXP)
                    if ct % 4 == 3:
                        ch = ct // 4
                        sl = slice(ch * CCH, (ch + 1) * CCH)
                        psr = ps_sc.tile([1, CCH], F32, tag="ps_sc")
                        mmr(psr, onecol, eqc[:, sl])
                        with nc.allow_low_precision(
                                reason="f32r==f32 bits; verifier plumbing"):
                            nc.vector.reciprocal(rrow[:, sl], psr)
                for ch in range(NCH):
                    sl = slice(ch * CCH, (ch + 1) * CCH)
                    # broadcast 1/r across partitions via K=1 matmul
                    psb = ps_sc.tile([128, CCH], F32, tag="ps_sc")
                    mmr(psb, ones_row, rrow[:, sl])
                    nc.vector.tensor_mul(s1t[:, sl], eqc[:, sl], psb)
                ps_t = ps_t_pool.tile([Q, EPAD], F32, tag="ps_t")
                for ct in range(NCT):
                    mmr(ps_t[:, 0:EPAD], ecq[:, ct, :], xcn[:, ct, :],
                        start=(ct == 0), stop=(ct == NCT - 1))
                # z' comes free from the ones column (200) of xcn
                rz = small.tile([Q, 1], F32, tag="rz")
                nc.vector.reciprocal(rz, ps_t[:, E:E + 1])
                t_sb = work.tile([Q, E], F32R, tag="t_sb")
                nc.vector.tensor_scalar_mul(t_sb, ps_t[:, 0:E], rz)

                # ---- c2q^T [e, c] and products ----
                c2qt0 = work.tile([128, C], F32R, tag="c2qt0")
                c2qt1 = work.tile([72, C], F32R, tag="c2qt1")
                p10 = work.tile([128, C], F32R, tag="p10")
                p11 = work.tile([72, C], F32R, tag="p11")
                p20 = work.tile([128, C], F32R, tag="p20")
                p21 = work.tile([72, C], F32R, tag="p21")
                for ch in range(NCH):
                    sl = slice(ch * CCH, (ch + 1) * CCH)
                    for ec, (e0, e1) in enumerate([(0, 128), (128, 200)]):
                        ne = e1 - e0
                        c2qt = (c2qt0, c2qt1)[ec]
                        p1 = (p10, p11)[ec]
                        xct_sl = xcta[:, sl] if ec == 0 else xctb[0:72, sl]
                        ps = ps_big.tile([128, CCH], F32, tag="ps_big")
                        mmr(ps[0:ne, :], xqn[:, e0:e1], s1t[:, sl])
                        if ec == 0:
                            nc.vector.tensor_copy(c2qt[:, sl], ps[0:ne, :])
                        else:
                            nc.scalar.copy(c2qt[:, sl], ps[0:ne, :])
                        # product on gpsimd (both SBUF) to offload DVE
                        nc.gpsimd.tensor_mul(p1[:, sl], c2qt[:, sl], xct_sl)
                    for ec, (e0, e1) in enumerate([(0, 128), (128, 200)]):
                        ne = e1 - e0
                        p2 = (p20, p21)[ec]
                        xct_sl = xcta[:, sl] if ec == 0 else xctb[0:72, sl]
                        # borrow the cq pool's banks (idle in this phase)
                        ps = ps_cq.tile([128, CCH], F32, tag="ps_cq")
                        mmr(ps[0:ne, :], t_sb[:, e0:e1], s1t[:, sl])
                        nc.vector.tensor_mul(p2[:, sl], ps[0:ne, :], xct_sl)

                # ---- projection out^T [o, c] ----
                feat_chunks = [xcta, xctb, c2qt0, c2qt1, p10, p11, p20, p21]
                out_sb = work.tile([O, C], F32, tag="out_sb")
                for ch in range(NCH):
                    sl = slice(ch * CCH, (ch + 1) * CCH)
                    pool = ps_big if ch % 2 == 0 else ps_cq
                    tagn = "ps_big" if ch % 2 == 0 else "ps_cq"
                    ps = pool.tile([128, CCH], F32, tag=tagn)
                    for k in range(8):
                        mmr(ps[0:O, :], wp_sb[k], feat_chunks[k][:, sl],
                            start=(k == 0), stop=(k == 7))
                    nc.scalar.copy(out_sb[:, sl], ps[0:O, :])
                    nc.sync.dma_start(out=d_out.ap()[b][:, sl],
                                      in_=out_sb[:, sl])

    nc.compile()
    return nc


def _get_nc():
    if "nc" not in _CACHE:
        _CACHE["nc"] = _build()
    return _CACHE["nc"]


def kernel(x_contexts, x_questions, w_sim, w_proj, b_proj, _trace=False):
    x_contexts = np.ascontiguousarray(x_contexts, dtype=np.float32)
    x_questions = np.ascontiguousarray(x_questions, dtype=np.float32)
    w_sim = np.asarray(w_sim, dtype=np.float32)
    w_proj = np.asarray(w_proj, dtype=np.float32)
    b_proj = np.asarray(b_proj, dtype=np.float32)

    # host-side layout prep (no model math)
    xct = np.ascontiguousarray(x_contexts.transpose(0, 2, 1))  # [B, E, C]
    xqt = np.ascontiguousarray(x_questions.transpose(0, 2, 1))  # [B, E, Q]
    xctb = np.empty((B, 73, C), np.float32)
    xctb[:, 0:72] = xct[:, 128:200]
    xctb[:, 72] = 1.0          # ones row (pairs the bias row of wp)
    xcn = np.zeros((B, NCT, 128, 204), np.float32)
    xcn[:, :, :, 0:E] = x_contexts.reshape(B, NCT, 128, E)
    xcn[:, :, :, E] = 1.0  # ones column: t-matmul accumulates z' there
    xq_pack = np.zeros((B, 128, 462), np.float32)
    w1, w2, w3 = w_sim[0, 0:200], w_sim[0, 200:400], w_sim[0, 400:600]
    xq_pack[:, :, 0], xq_pack[:, 0:72, 1] = w1[0:128], w1[128:200]
    xq_pack[:, :, 2], xq_pack[:, 0:72, 3] = w2[0:128], w2[128:200]
    xq_pack[:, :, 4], xq_pack[:, 0:72, 5] = w3[0:128], w3[128:200]
    xq_pack[:, :, 6:134] = xqt[:, 0:128]
    xq_pack[:, 0:72, 134:262] = xqt[:, 128:200, :]
    xq_pack[:, :, 262:462] = x_questions
    wpfull = np.concatenate(
        [w_proj.T[0:200], b_proj[None, :], w_proj.T[200:800]], axis=0)
    WP_CH = [(0, 128), (128, 201), (201, 329), (329, 401),
             (401, 529), (529, 601), (601, 729), (729, 801)]
    wp = np.zeros((8, 128, O), np.float32)
    for k, (r0, r1) in enumerate(WP_CH):
        wp[k, 0:r1 - r0] = wpfull[r0:r1]

    in_maps = []
    for c in range(NCORES):
        bs = slice(c * NB, (c + 1) * NB)
        in_maps.append({
            "xcta": np.ascontiguousarray(xct[bs, 0:128]),
            "xctb": np.ascontiguousarray(xctb[bs]),
            "xcn": np.ascontiguousarray(xcn[bs]),
            "xq": np.ascontiguousarray(xq_pack[bs]),
            "wp": wp,
        })

    nc = _get_nc()
    res = run_bass_kernel_spmd(nc, in_maps, core_ids=list(range(NCORES)),
                               trace=_trace)
    _CACHE["last_res"] = res

    out = np.empty((B, C, O), np.float32)
    for c in range(NCORES):
        ot = res.results[c]["out_t"]  # [NB, O, C]
        for b in range(NB):
            out[c * NB + b] = ot[b].T
    return out



# revision 7
# speedup vs baseline: 1.0696x; 1.0696x over previous
"""BiDAF-style attention kernel for Trainium2, data-parallel over batch on 8 cores.

Shapes (hardcoded): B=16, C=2048, Q=128, E=200, O=128. Each core: 2 batches.

v2 design (bf16 everywhere, fp32 psum accumulation):
- Scores computed once, in cq-orientation ([c-part, q]) with bf16 matmuls
  (N=128 runs at 1 cycle/row in bf16; f32r would need N>=256).
  s_c folds into the rhs via lhsq = w3*xqT + w1; s_q rides a ones-row of
  xctb paired with an s_q row of lhsq_b (both placed at partition 64 so the
  s_q [1,128] psum->sbuf copy is partition-aligned).
- Softmax-q denominator r comes FREE from the exp's accum_out. 1/r is a
  per-partition scalar in cq-layout, so S1 = E*rinv needs no broadcast
  matmuls. S1^T is produced by a DMA xbar transpose (14ns/16x128-tile),
  eliminating the entire second score computation of v1.
- Softmax-c side: t = S2^T Xc via ecq-lhsT matmuls against natural-layout
  context tiles (ones column gives z). exp(s_c)/exp(s_q) factors cancel in
  the respective normalizations, so full scores are correct for both.
- c2q's projection goes through Y2 = Xq W2p^T (Q=O=128): W2-chunks of the
  projection collapse from 2 matmuls to 1 per chunk; c2q itself is only
  needed for the product p1 and is consumed straight from PSUM.
- Projection output DMAs straight from PSUM to DRAM (f32), no copy.
"""

import numpy as np
import ml_dtypes

import concourse.bass as bass
import concourse.mybir as mybir
from concourse import bacc
from concourse.bass import MemorySpace
from concourse.tile import TileContext
from concourse.bass_utils import run_bass_kernel_spmd

B, C, Q, E, O = 16, 2048, 128, 200, 128
NB = 2
NCORES = 8
NCT = 16          # c tiles of 128
F32 = mybir.dt.float32
F16 = mybir.dt.float16
BF = mybir.dt.bfloat16
EXP = mybir.ActivationFunctionType.Exp
MUL = mybir.AluOpType.mult
ADD = mybir.AluOpType.add

_CACHE = {}


def _build(num_devices=NCORES, reps=1):
    nc = bacc.Bacc("TRN2", target_bir_lowering=False, debug=False,
                   num_devices=num_devices)

    d_xcta = nc.dram_tensor("xcta", [NB, 128, C], BF, kind="ExternalInput")
    d_xctb = nc.dram_tensor("xctb", [NB, 73, C], BF, kind="ExternalInput")
    # natural-layout ctx tiles + ones col: [p, ct*201+j]
    d_xcn = nc.dram_tensor("xcn", [NB, 128, NCT * 201], BF,
                           kind="ExternalInput")
    d_xq = nc.dram_tensor("xq", [NB, 128, 461], BF, kind="ExternalInput")
    d_wc = nc.dram_tensor("wc", [128, 2], F32, kind="ExternalInput")
    d_wp = nc.dram_tensor("wp", [8, 128, O], BF, kind="ExternalInput")
    d_out = nc.dram_tensor("out_t", [NB, O, C], F16, kind="ExternalOutput")

    def mm(ps, lhsT, rhs, start=True, stop=True):
        nc.tensor.matmul(ps, lhsT, rhs, start=start, stop=stop)

    with TileContext(nc) as tc:
        with (
            tc.tile_pool(name="consts", bufs=1) as consts,
            tc.tile_pool(name="inputs", bufs=2) as inputs,
            tc.tile_pool(name="work", bufs=2) as work,
            tc.tile_pool(name="work1", bufs=1) as work1,
            tc.tile_pool(name="ps_cq", bufs=2, space=MemorySpace.PSUM) as ps_cq,
            tc.tile_pool(name="wa", bufs=4, space=MemorySpace.PSUM) as wa,
            tc.tile_pool(name="po", bufs=2, space=MemorySpace.PSUM) as po_pool,
        ):
            wp_all = consts.tile([128, 8, O], BF, tag="wp")
            wc = consts.tile([128, 2], F32, tag="wc")

            for rep in range(reps):
                # ---- all input DMAs up front (SP queue stays unblocked) ----
                xqs, xctas, xctbs, xcns = [], [], [], []
                for b in range(NB):
                    xq = inputs.tile([128, 461], BF, tag="xq")
                    nc.sync.dma_start(out=xq, in_=d_xq.ap()[b])
                    if rep == 0 and b == 0:
                        nc.sync.dma_start(
                            out=wp_all,
                            in_=d_wp.ap().rearrange("k p o -> p k o"))
                        nc.sync.dma_start(out=wc, in_=d_wc.ap())
                    xcta = inputs.tile([128, C], BF, tag="xcta")
                    xctb = inputs.tile([73, C], BF, tag="xctb")
                    xcn = inputs.tile([128, NCT, 201], BF, tag="xcn")
                    for h in range(2):
                        csl = slice(h * 1024, (h + 1) * 1024)
                        nc.sync.dma_start(out=xcta[:, csl],
                                          in_=d_xcta.ap()[b][:, csl])
                        nc.sync.dma_start(out=xctb[:, csl],
                                          in_=d_xctb.ap()[b][:, csl])
                        tsl = slice(h * 8, (h + 1) * 8)
                        nc.sync.dma_start(
                            out=xcn[:, tsl, :],
                            in_=d_xcn.ap()[b][:, h * 1608:(h + 1) * 1608])
                    xqs.append(xq)
                    xctas.append(xcta)
                    xctbs.append(xctb)
                    xcns.append(xcn)

                # ---- phase 1 (both batches): scores, softmax pieces, t ----
                y2s, s1ts, tsbs, ps_ts = [], [], [], []
                for b in range(NB):
                    xq, xcta, xctb, xcn = xqs[b], xctas[b], xctbs[b], xcns[b]
                    xqta = xq[:, 0:128]
                    xqtb = xq[0:73, 128:256]
                    lhsq_a = work.tile([128, 128], BF, tag="lhsq_a")
                    nc.vector.scalar_tensor_tensor(
                        lhsq_a, xqta, wc[:, 0:1],
                        xq[:, 459:460].broadcast_to([128, 128]),
                        op0=MUL, op1=ADD)
                    lhsq_b = work.tile([73, 128], BF, tag="lhsq_b")
                    nc.vector.scalar_tensor_tensor(
                        lhsq_b, xqtb, wc[0:73, 1:2],
                        xq[0:73, 460:461].broadcast_to([73, 128]),
                        op0=MUL, op1=ADD)
                    # s_q row -> psum partition 64 -> lhsq_b row 64
                    ps_sq = ps_cq.tile([128, 128], F32, tag="cq")
                    mm(ps_sq[64:65, :], xq[:, 457:458], xqta,
                       start=True, stop=False)
                    mm(ps_sq[64:65, :], xq[0:73, 458:459], xqtb,
                       start=False, stop=True)
                    nc.vector.tensor_copy(lhsq_b[64:65, :], ps_sq[64:65, :])
                    # Y2 = Xq @ W2p^T  [q, o]
                    ps_y2 = ps_cq.tile([128, 128], F32, tag="cq")
                    mm(ps_y2, xqta, wp_all[:, 2, :], start=True, stop=False)
                    mm(ps_y2, xqtb, wp_all[0:73, 3, :], start=False, stop=True)
                    y2 = work.tile([128, 128], BF, tag="y2")
                    nc.scalar.copy(y2, ps_y2)

                    ecq = work.tile([128, NCT, 128], BF, tag="ecq")
                    rcol = work.tile([128, NCT], F32, tag="rcol")
                    ps_t = wa.tile([128, 512], F32, tag="wa")
                    for ct in range(NCT):
                        csl = slice(ct * 128, (ct + 1) * 128)
                        ps = ps_cq.tile([128, 128], F32, tag="cq")
                        mm(ps, xcta[:, csl], lhsq_a, start=True, stop=False)
                        mm(ps, xctb[:, csl], lhsq_b, start=False, stop=True)
                        nc.scalar.activation(
                            out=ecq[:, ct, :], in_=ps, func=EXP,
                            accum_out=rcol[:, ct:ct + 1])
                        mm(ps_t[:, 0:201], ecq[:, ct, :], xcn[:, ct, :],
                           start=(ct == 0), stop=(ct == NCT - 1))

                    rinv = work.tile([128, NCT, 1], F32, tag="rinv")
                    nc.vector.reciprocal(rinv[:, :, 0], rcol)
                    s1 = work.tile([128, NCT, 128], BF, tag="s1")
                    nc.vector.tensor_mul(
                        s1, ecq, rinv.broadcast_to([128, NCT, 128]))
                    s1t = work.tile([128, NCT, 128], BF, tag="s1t")
                    for h in range(2):
                        tsl = slice(h * 8, (h + 1) * 8)
                        nc.sync.dma_start_transpose(
                            out=s1t[:, tsl, :], in_=s1[:, tsl, :])
                    # t = S2^T Xc (x exp(s_q) factor; cancels via z col 200)
                    rz = work.tile([128, 1], F32, tag="rz")
                    nc.vector.reciprocal(rz, ps_t[:, 200:201])
                    tsb = work.tile([128, 224], BF, tag="tsb")
                    nc.vector.memset(tsb[:, 192:193], 0.0)
                    nc.vector.tensor_scalar_mul(
                        tsb[:, 0:192], ps_t[:, 0:192], rz)
                    nc.vector.tensor_scalar_mul(
                        tsb[:, 193:201], ps_t[:, 192:200], rz)
                    y2s.append(y2)
                    s1ts.append(s1t)
                    tsbs.append(tsb)

                # ---- phase 2 (both batches): c2q, q2c, products, proj ----
                for b in range(NB):
                    xq, xcta, xctb = xqs[b], xctas[b], xctbs[b]
                    y2, s1t, tsb = y2s[b], s1ts[b], tsbs[b]
                    p1a = work1.tile([128, C], BF, tag="p1a")
                    p1b = work1.tile([73, C], BF, tag="p1b")
                    p2a = work1.tile([128, C], BF, tag="p2a")
                    p2b = work1.tile([73, C], BF, tag="p2b")
                    q2a = work1.tile([128, C], BF, tag="q2a")
                    q2b = work1.tile([73, C], BF, tag="q2b")
                    out_sb = work1.tile([O, C], F16, tag="out_sb")

                    def emit_proj(ch):
                        csl = slice(ch * 512, (ch + 1) * 512)
                        s1t_ch = s1t[:, 4 * ch:4 * ch + 4, :]
                        pp = po_pool.tile([128, 512], F32, tag="po")
                        mm(pp, wp_all[:, 0, :], xcta[:, csl],
                           start=True, stop=False)
                        mm(pp, wp_all[0:73, 1, :], xctb[:, csl],
                           start=False, stop=False)
                        mm(pp, y2, s1t_ch, start=False, stop=False)
                        mm(pp, wp_all[:, 4, :], p1a[:, csl],
                           start=False, stop=False)
                        mm(pp, wp_all[0:73, 5, :], p1b[0:73, csl],
                           start=False, stop=False)
                        mm(pp, wp_all[:, 6, :], p2a[:, csl],
                           start=False, stop=False)
                        mm(pp, wp_all[0:73, 7, :], p2b[0:73, csl],
                           start=False, stop=True)
                        if ch % 2 == 0:
                            nc.scalar.copy(out_sb[:, csl], pp)
                        else:
                            nc.vector.tensor_copy(out_sb[:, csl], pp)
                        nc.sync.dma_start(out=d_out.ap()[b][:, csl],
                                          in_=out_sb[:, csl])

                    for ch in range(4):
                        csl = slice(ch * 512, (ch + 1) * 512)
                        s1t_ch = s1t[:, 4 * ch:4 * ch + 4, :]
                        pa1 = wa.tile([128, 512], F32, tag="wa")
                        mm(pa1, xq[:, 256:384], s1t_ch)
                        pb1 = wa.tile([128, 512], F32, tag="wa")
                        mm(pb1[0:73, :], xq[:, 384:457], s1t_ch)
                        pa2 = wa.tile([128, 512], F32, tag="wa")
                        mm(pa2, tsb[:, 0:128], s1t_ch)
                        pb2 = wa.tile([128, 512], F32, tag="wa")
                        mm(pb2[0:73, :], tsb[:, 128:201], s1t_ch)
                        # p1 = xct * c2q^T straight from psum (DVE)
                        nc.vector.tensor_mul(p1a[:, csl], pa1, xcta[:, csl])
                        nc.vector.tensor_mul(p1b[0:73, csl], pb1[0:73, :],
                                             xctb[:, csl])
                        # p2 = xct * q2c^T: ACT copy to bf16, then pool TT
                        nc.scalar.copy(q2a[:, csl], pa2)
                        nc.scalar.copy(q2b[0:73, csl], pb2[0:73, :])
                        nc.gpsimd.tensor_mul(p2a[:, csl], q2a[:, csl],
                                             xcta[:, csl])
                        nc.gpsimd.tensor_mul(p2b[0:73, csl], q2b[0:73, csl],
                                             xctb[:, csl])
                        if ch > 0:
                            emit_proj(ch - 1)
                    emit_proj(3)

    nc.compile()
    return nc


def _get_nc():
    if "nc" not in _CACHE:
        _CACHE["nc"] = _build()
    return _CACHE["nc"]


def _pack_rearranged(dst, src, row64=None):
    """dst rows 0:64 = src rows 0:64; row 64 = row64 (or 0); 65:73 = src 64:72."""
    dst[0:64] = src[0:64]
    if row64 is not None:
        dst[64] = row64
    dst[65:73] = src[64:72]


def kernel(x_contexts, x_questions, w_sim, w_proj, b_proj, _trace=False):
    bf16 = ml_dtypes.bfloat16
    x_contexts = np.ascontiguousarray(x_contexts, dtype=np.float32)
    x_questions = np.ascontiguousarray(x_questions, dtype=np.float32)
    w_sim = np.asarray(w_sim, dtype=np.float32)
    w_proj = np.asarray(w_proj, dtype=np.float32)
    b_proj = np.asarray(b_proj, dtype=np.float32)
    w1, w2, w3 = w_sim[0, 0:E], w_sim[0, E:2 * E], w_sim[0, 2 * E:]

    xct = x_contexts.transpose(0, 2, 1)            # [B, E, C]
    xcta = np.ascontiguousarray(xct[:, 0:128]).astype(bf16)
    xctb = np.zeros((B, 73, C), np.float32)
    for bi in range(B):
        _pack_rearranged(xctb[bi], xct[bi, 128:200], row64=1.0)
    xctb = xctb.astype(bf16)
    xcn = np.zeros((B, 128, NCT, 201), np.float32)
    xcn[:, :, :, 0:E] = x_contexts.reshape(B, NCT, 128, E).transpose(0, 2, 1, 3)
    xcn[:, :, :, E] = 1.0
    xcn = xcn.reshape(B, 128, NCT * 201).astype(bf16)

    xqt = x_questions.transpose(0, 2, 1)           # [B, E, Q]
    xq = np.zeros((B, 128, 461), np.float32)
    xq[:, :, 0:128] = xqt[:, 0:128]
    xq[:, 0:64, 128:256] = xqt[:, 128:192]
    xq[:, 65:73, 128:256] = xqt[:, 192:200]
    xq[:, :, 256:384] = x_questions[:, :, 0:128]
    xq[:, :, 384:448] = x_questions[:, :, 128:192]
    xq[:, :, 448] = 0.0
    xq[:, :, 449:457] = x_questions[:, :, 192:200]
    xq[:, 0:128, 457] = w2[0:128]
    xq[:, 0:64, 458] = w2[128:192]
    xq[:, 65:73, 458] = w2[192:200]
    xq[:, 0:128, 459] = w1[0:128]
    xq[:, 0:64, 460] = w1[128:192]
    xq[:, 65:73, 460] = w1[192:200]
    xq = xq.astype(bf16)

    wc = np.zeros((128, 2), np.float32)
    wc[0:128, 0] = w3[0:128]
    wc[0:64, 1] = w3[128:192]
    wc[65:73, 1] = w3[192:200]

    wpT = w_proj.T                                 # [800, O]
    wp = np.zeros((8, 128, O), np.float32)
    wp[0] = wpT[0:128]                             # W1 e0:128
    _pack_rearranged(wp[1], wpT[128:200], row64=None)
    wp[1, 64] = b_proj                             # bias pairs the ones row
    wp[2] = wpT[200:328]                           # W2^T e0:128 (Y2 rhs)
    _pack_rearranged(wp[3], wpT[328:400])
    wp[4] = wpT[400:528]                           # W3 e0:128
    _pack_rearranged(wp[5], wpT[528:600])
    wp[6] = wpT[600:728]                           # W4 e0:128
    _pack_rearranged(wp[7], wpT[728:800])
    wp = wp.astype(bf16)

    in_maps = []
    for c in range(NCORES):
        bs = slice(c * NB, (c + 1) * NB)
        in_maps.append({
            "xcta": np.ascontiguousarray(xcta[bs]),
            "xctb": np.ascontiguousarray(xctb[bs]),
            "xcn": np.ascontiguousarray(xcn[bs]),
            "xq": np.ascontiguousarray(xq[bs]),
            "wc": wc,
            "wp": wp,
        })

    nc = _get_nc()
    res = run_bass_kernel_spmd(nc, in_maps, core_ids=list(range(NCORES)),
                               trace=_trace)
    _CACHE["last_res"] = res

    out = np.empty((B, C, O), np.float32)
    for c in range(NCORES):
        ot = res.results[c]["out_t"]               # [NB, O, C] f32
        for b in range(NB):
            out[c * NB + b] = np.asarray(ot[b], dtype=np.float32).T
    return out


# revision 12
# speedup vs baseline: 1.1727x; 1.0964x over previous
"""BiDAF-style attention kernel for Trainium2, data-parallel over batch on 8 cores.

Shapes (hardcoded): B=16, C=2048, Q=128, E=200, O=128. Each core: 2 batches.

v2 design (bf16 everywhere, fp32 psum accumulation):
- Scores computed once, in cq-orientation ([c-part, q]) with bf16 matmuls
  (N=128 runs at 1 cycle/row in bf16; f32r would need N>=256).
  s_c folds into the rhs via lhsq = w3*xqT + w1; s_q rides a ones-row of
  xctb paired with an s_q row of lhsq_b (both placed at partition 64 so the
  s_q [1,128] psum->sbuf copy is partition-aligned).
- Softmax-q denominator r comes FREE from the exp's accum_out. 1/r is a
  per-partition scalar in cq-layout, so S1 = E*rinv needs no broadcast
  matmuls. S1^T is produced by a DMA xbar transpose (14ns/16x128-tile),
  eliminating the entire second score computation of v1.
- Softmax-c side: t = S2^T Xc via ecq-lhsT matmuls against natural-layout
  context tiles (ones column gives z). exp(s_c)/exp(s_q) factors cancel in
  the respective normalizations, so full scores are correct for both.
- c2q's projection goes through Y2 = Xq W2p^T (Q=O=128): W2-chunks of the
  projection collapse from 2 matmuls to 1 per chunk; c2q itself is only
  needed for the product p1 and is consumed straight from PSUM.
- Projection output DMAs straight from PSUM to DRAM (f32), no copy.
"""

import numpy as np
import ml_dtypes

import concourse.bass as bass
import concourse.mybir as mybir
from concourse import bacc
from concourse.bass import MemorySpace
from concourse.tile import TileContext
from concourse.bass_utils import run_bass_kernel_spmd

B, C, Q, E, O = 16, 2048, 128, 200, 128
NB = 2
NCORES = 8
NCT = 16          # c tiles of 128
F32 = mybir.dt.float32
F16 = mybir.dt.float16
BF = mybir.dt.bfloat16
EXP = mybir.ActivationFunctionType.Exp
MUL = mybir.AluOpType.mult
ADD = mybir.AluOpType.add

_CACHE = {}


def _build(num_devices=NCORES, reps=1):
    nc = bacc.Bacc("TRN2", target_bir_lowering=False, debug=False,
                   num_devices=num_devices)

    d_xcta = nc.dram_tensor("xcta", [NB, 128, C], BF, kind="ExternalInput")
    d_xctb = nc.dram_tensor("xctb", [NB, 73, C], BF, kind="ExternalInput")
    # natural-layout ctx tiles + ones col: [p, ct*201+j]
    d_xcn = nc.dram_tensor("xcn", [NB, 128, NCT * 201], BF,
                           kind="ExternalInput")
    d_xq = nc.dram_tensor("xq", [NB, 128, 461], BF, kind="ExternalInput")
    d_wc = nc.dram_tensor("wc", [128, 2], F32, kind="ExternalInput")
    d_wp = nc.dram_tensor("wp", [8, 128, O], BF, kind="ExternalInput")
    d_out = nc.dram_tensor("out_t", [NB, O, C], F16, kind="ExternalOutput")

    def mm(ps, lhsT, rhs, start=True, stop=True):
        nc.tensor.matmul(ps, lhsT, rhs, start=start, stop=stop)

    with TileContext(nc) as tc:
        with (
            tc.tile_pool(name="consts", bufs=1) as consts,
            tc.tile_pool(name="inputs", bufs=2) as inputs,
            tc.tile_pool(name="work", bufs=2) as work,
            tc.tile_pool(name="work1", bufs=1) as work1,
            tc.tile_pool(name="cqw", bufs=2, space=MemorySpace.PSUM) as ps_cqw,
            tc.tile_pool(name="pst", bufs=1, space=MemorySpace.PSUM) as ps_tp,
            tc.tile_pool(name="wa", bufs=4, space=MemorySpace.PSUM) as wa,
            tc.tile_pool(name="po", bufs=1, space=MemorySpace.PSUM) as po_pool,
        ):
            wp_all = consts.tile([128, 8, O], BF, tag="wp")
            wc = consts.tile([128, 2], F32, tag="wc")

            for rep in range(reps):
                # ---- all input DMAs up front (SP queue stays unblocked) ----
                xqs, xctas, xctbs, xcns = [], [], [], []
                for b in range(NB):
                    xq = inputs.tile([128, 461], BF, tag="xq")
                    nc.sync.dma_start(out=xq, in_=d_xq.ap()[b])
                    if rep == 0 and b == 0:
                        nc.sync.dma_start(
                            out=wp_all,
                            in_=d_wp.ap().rearrange("k p o -> p k o"))
                        nc.sync.dma_start(out=wc, in_=d_wc.ap())
                    xcta = inputs.tile([128, C], BF, tag="xcta")
                    xctb = inputs.tile([73, C], BF, tag="xctb")
                    xcn = inputs.tile([128, NCT, 201], BF, tag="xcn")
                    for h in range(2):
                        csl = slice(h * 1024, (h + 1) * 1024)
                        nc.sync.dma_start(out=xcta[:, csl],
                                          in_=d_xcta.ap()[b][:, csl])
                        nc.sync.dma_start(out=xctb[:, csl],
                                          in_=d_xctb.ap()[b][:, csl])
                        tsl = slice(h * 8, (h + 1) * 8)
                        nc.sync.dma_start(
                            out=xcn[:, tsl, :],
                            in_=d_xcn.ap()[b][:, h * 1608:(h + 1) * 1608])
                    xqs.append(xq)
                    xctas.append(xcta)
                    xctbs.append(xctb)
                    xcns.append(xcn)

                # ---- phase 1 (both batches): scores, softmax pieces, t ----
                y2s, s1ts, tsbs, ps_ts = [], [], [], []
                for b in range(NB):
                    xq, xcta, xctb, xcn = xqs[b], xctas[b], xctbs[b], xcns[b]
                    xqta = xq[:, 0:128]
                    xqtb = xq[0:73, 128:256]
                    lhsq_a = work.tile([128, 128], BF, tag="lhsq_a")
                    nc.vector.scalar_tensor_tensor(
                        lhsq_a, xqta, wc[:, 0:1],
                        xq[:, 459:460].broadcast_to([128, 128]),
                        op0=MUL, op1=ADD)
                    lhsq_b = work.tile([73, 128], BF, tag="lhsq_b")
                    nc.vector.scalar_tensor_tensor(
                        lhsq_b, xqtb, wc[0:73, 1:2],
                        xq[0:73, 460:461].broadcast_to([73, 128]),
                        op0=MUL, op1=ADD)
                    # s_q row -> psum partition 64 -> lhsq_b row 64
                    ps_sq = ps_cqw.tile([128, 512], F32, tag="cqw")
                    mm(ps_sq[64:65, 0:128], xq[:, 457:458], xqta,
                       start=True, stop=False)
                    mm(ps_sq[64:65, 0:128], xq[0:73, 458:459], xqtb,
                       start=False, stop=True)
                    nc.vector.tensor_copy(lhsq_b[64:65, :],
                                          ps_sq[64:65, 0:128])
                    # Y2 = Xq @ W2p^T  [q, o]
                    ps_y2 = ps_cqw.tile([128, 512], F32, tag="cqw")
                    mm(ps_y2[:, 0:128], xqta, wp_all[:, 2, :],
                       start=True, stop=False)
                    mm(ps_y2[:, 0:128], xqtb, wp_all[0:73, 3, :],
                       start=False, stop=True)
                    y2 = work.tile([128, 128], BF, tag="y2")
                    nc.scalar.copy(y2, ps_y2[:, 0:128])

                    ecq = work.tile([128, NCT, 128], BF, tag="ecq")
                    rcol = work.tile([128, NCT], F32, tag="rcol")
                    rinv = work.tile([128, NCT, 1], F32, tag="rinv")
                    s1 = work.tile([128, NCT, 128], BF, tag="s1")
                    s1t = work.tile([128, NCT, 128], BF, tag="s1t")
                    ps_t = ps_tp.tile([128, 512], F32, tag="pst")

                    def s1_half(h):
                        tsl = slice(h * 8, (h + 1) * 8)
                        nc.vector.reciprocal(rinv[:, tsl, 0], rcol[:, tsl])
                        nc.gpsimd.tensor_mul(
                            s1[:, tsl, :], ecq[:, tsl, :],
                            rinv[:, tsl, :].broadcast_to([128, 8, 128]))
                        nc.sync.dma_start_transpose(
                            out=s1t[:, tsl, :], in_=s1[:, tsl, :])

                    for g in range(4):
                        gsl = slice(g * 4, (g + 1) * 4)
                        ps = ps_cqw.tile([128, 512], F32, tag="cqw")
                        for k in range(4):
                            ct = 4 * g + k
                            csl = slice(ct * 128, (ct + 1) * 128)
                            ksl = slice(k * 128, (k + 1) * 128)
                            mm(ps[:, ksl], xcta[:, csl], lhsq_a,
                               start=True, stop=False)
                            mm(ps[:, ksl], xctb[:, csl], lhsq_b,
                               start=False, stop=True)
                        nc.scalar.activation(
                            out=ecq[:, gsl, :], in_=ps, func=EXP)
                        nc.vector.reduce_sum(
                            rcol[:, gsl], ecq[:, gsl, :],
                            axis=mybir.AxisListType.X)
                        for k in range(4):
                            ct = 4 * g + k
                            mm(ps_t[:, 0:201], ecq[:, ct, :], xcn[:, ct, :],
                               start=(ct == 0), stop=(ct == NCT - 1))
                        if g == 1:
                            s1_half(0)
                    s1_half(1)
                    # t = S2^T Xc (x exp(s_q) factor; cancels via z col 200)
                    rz = work.tile([128, 1], F32, tag="rz")
                    nc.vector.reciprocal(rz, ps_t[:, 200:201])
                    tsb = work.tile([128, 224], BF, tag="tsb")
                    nc.vector.memset(tsb[:, 192:193], 0.0)
                    nc.vector.tensor_scalar_mul(
                        tsb[:, 0:192], ps_t[:, 0:192], rz)
                    nc.vector.tensor_scalar_mul(
                        tsb[:, 193:201], ps_t[:, 192:200], rz)
                    y2s.append(y2)
                    s1ts.append(s1t)
                    tsbs.append(tsb)

                # ---- phase 2 (both batches): c2q, q2c, products, proj ----
                for b in range(NB):
                    xq, xcta, xctb = xqs[b], xctas[b], xctbs[b]
                    y2, s1t, tsb = y2s[b], s1ts[b], tsbs[b]
                    p1a = work1.tile([128, C], BF, tag="p1a")
                    p1b = work1.tile([73, C], BF, tag="p1b")
                    p2a = work1.tile([128, C], BF, tag="p2a")
                    p2b = work1.tile([73, C], BF, tag="p2b")
                    q2b = work1.tile([73, C], BF, tag="q2b")
                    out_sb = work1.tile([O, C], F16, tag="out_sb")

                    def emit_proj(ch):
                        csl = slice(ch * 512, (ch + 1) * 512)
                        s1t_ch = s1t[:, 4 * ch:4 * ch + 4, :]
                        pp = po_pool.tile([128, 512], F32, tag="po")
                        mm(pp, wp_all[:, 0, :], xcta[:, csl],
                           start=True, stop=False)
                        mm(pp, wp_all[0:73, 1, :], xctb[:, csl],
                           start=False, stop=False)
                        mm(pp, y2, s1t_ch, start=False, stop=False)
                        mm(pp, wp_all[:, 4, :], p1a[:, csl],
                           start=False, stop=False)
                        mm(pp, wp_all[0:73, 5, :], p1b[0:73, csl],
                           start=False, stop=False)
                        mm(pp, wp_all[:, 6, :], p2a[:, csl],
                           start=False, stop=False)
                        mm(pp, wp_all[0:73, 7, :], p2b[0:73, csl],
                           start=False, stop=True)
                        nc.scalar.copy(out_sb[:, csl], pp)
                        nc.sync.dma_start(out=d_out.ap()[b][:, csl],
                                          in_=out_sb[:, csl])

                    for ch in range(4):
                        csl = slice(ch * 512, (ch + 1) * 512)
                        s1t_ch = s1t[:, 4 * ch:4 * ch + 4, :]
                        pa1 = wa.tile([128, 512], F32, tag="wa")
                        mm(pa1, xq[:, 256:384], s1t_ch)
                        pb1 = wa.tile([128, 512], F32, tag="wa")
                        mm(pb1[0:73, :], xq[:, 384:457], s1t_ch)
                        pa2 = wa.tile([128, 512], F32, tag="wa")
                        mm(pa2, tsb[:, 0:128], s1t_ch)
                        pb2 = wa.tile([128, 512], F32, tag="wa")
                        mm(pb2[0:73, :], tsb[:, 128:201], s1t_ch)
                        # p1/p2a straight from psum on DVE; p2b via ACT
                        # copy + pool TT (pool is idle in phase 2 otherwise)
                        nc.vector.tensor_mul(p1a[:, csl], pa1, xcta[:, csl])
                        nc.vector.tensor_mul(p1b[0:73, csl], pb1[0:73, :],
                                             xctb[:, csl])
                        nc.vector.tensor_mul(p2a[:, csl], pa2, xcta[:, csl])
                        nc.scalar.copy(q2b[0:73, csl], pb2[0:73, :])
                        nc.gpsimd.tensor_mul(p2b[0:73, csl], q2b[0:73, csl],
                                             xctb[:, csl])
                        if ch > 0:
                            emit_proj(ch - 1)
                    emit_proj(3)

    nc.compile()
    return nc


def _get_nc():
    if "nc" not in _CACHE:
        _CACHE["nc"] = _build()
    return _CACHE["nc"]


def _pack_rearranged(dst, src, row64=None):
    """dst rows 0:64 = src rows 0:64; row 64 = row64 (or 0); 65:73 = src 64:72."""
    dst[0:64] = src[0:64]
    if row64 is not None:
        dst[64] = row64
    dst[65:73] = src[64:72]


def kernel(x_contexts, x_questions, w_sim, w_proj, b_proj, _trace=False):
    bf16 = ml_dtypes.bfloat16
    x_contexts = np.ascontiguousarray(x_contexts, dtype=np.float32)
    x_questions = np.ascontiguousarray(x_questions, dtype=np.float32)
    w_sim = np.asarray(w_sim, dtype=np.float32)
    w_proj = np.asarray(w_proj, dtype=np.float32)
    b_proj = np.asarray(b_proj, dtype=np.float32)
    w1, w2, w3 = w_sim[0, 0:E], w_sim[0, E:2 * E], w_sim[0, 2 * E:]

    xct = x_contexts.transpose(0, 2, 1)            # [B, E, C]
    xcta = np.ascontiguousarray(xct[:, 0:128]).astype(bf16)
    xctb = np.zeros((B, 73, C), np.float32)
    for bi in range(B):
        _pack_rearranged(xctb[bi], xct[bi, 128:200], row64=1.0)
    xctb = xctb.astype(bf16)
    xcn = np.zeros((B, 128, NCT, 201), np.float32)
    xcn[:, :, :, 0:E] = x_contexts.reshape(B, NCT, 128, E).transpose(0, 2, 1, 3)
    xcn[:, :, :, E] = 1.0
    xcn = xcn.reshape(B, 128, NCT * 201).astype(bf16)

    xqt = x_questions.transpose(0, 2, 1)           # [B, E, Q]
    xq = np.zeros((B, 128, 461), np.float32)
    xq[:, :, 0:128] = xqt[:, 0:128]
    xq[:, 0:64, 128:256] = xqt[:, 128:192]
    xq[:, 65:73, 128:256] = xqt[:, 192:200]
    xq[:, :, 256:384] = x_questions[:, :, 0:128]
    xq[:, :, 384:448] = x_questions[:, :, 128:192]
    xq[:, :, 448] = 0.0
    xq[:, :, 449:457] = x_questions[:, :, 192:200]
    xq[:, 0:128, 457] = w2[0:128]
    xq[:, 0:64, 458] = w2[128:192]
    xq[:, 65:73, 458] = w2[192:200]
    xq[:, 0:128, 459] = w1[0:128]
    xq[:, 0:64, 460] = w1[128:192]
    xq[:, 65:73, 460] = w1[192:200]
    xq = xq.astype(bf16)

    wc = np.zeros((128, 2), np.float32)
    wc[0:128, 0] = w3[0:128]
    wc[0:64, 1] = w3[128:192]
    wc[65:73, 1] = w3[192:200]

    wpT = w_proj.T                                 # [800, O]
    wp = np.zeros((8, 128, O), np.float32)
    wp[0] = wpT[0:128]                             # W1 e0:128
    _pack_rearranged(wp[1], wpT[128:200], row64=None)
    wp[1, 64] = b_proj                             # bias pairs the ones row
    wp[2] = wpT[200:328]                           # W2^T e0:128 (Y2 rhs)
    _pack_rearranged(wp[3], wpT[328:400])
    wp[4] = wpT[400:528]                           # W3 e0:128
    _pack_rearranged(wp[5], wpT[528:600])
    wp[6] = wpT[600:728]                           # W4 e0:128
    _pack_rearranged(wp[7], wpT[728:800])
    wp = wp.astype(bf16)

    in_maps = []
    for c in range(NCORES):
        bs = slice(c * NB, (c + 1) * NB)
        in_maps.append({
            "xcta": np.ascontiguousarray(xcta[bs]),
            "xctb": np.ascontiguousarray(xctb[bs]),
            "xcn": np.ascontiguousarray(xcn[bs]),
            "xq": np.ascontiguousarray(xq[bs]),
            "wc": wc,
            "wp": wp,
        })

    nc = _get_nc()
    res = run_bass_kernel_spmd(nc, in_maps, core_ids=list(range(NCORES)),
                               trace=_trace)
    _CACHE["last_res"] = res

    out = np.empty((B, C, O), np.float32)
    for c in range(NCORES):
        ot = res.results[c]["out_t"]               # [NB, O, C] f32
        for b in range(NB):
            out[c * NB + b] = np.asarray(ot[b], dtype=np.float32).T
    return out


# revision 15
# speedup vs baseline: 1.1853x; 1.0107x over previous
"""BiDAF-style attention kernel for Trainium2, data-parallel over batch on 8 cores.

Shapes (hardcoded): B=16, C=2048, Q=128, E=200, O=128. Each core: 2 batches.

v2 design (bf16 everywhere, fp32 psum accumulation):
- Scores computed once, in cq-orientation ([c-part, q]) with bf16 matmuls
  (N=128 runs at 1 cycle/row in bf16; f32r would need N>=256).
  s_c folds into the rhs via lhsq = w3*xqT + w1; s_q rides a ones-row of
  xctb paired with an s_q row of lhsq_b (both placed at partition 64 so the
  s_q [1,128] psum->sbuf copy is partition-aligned).
- Softmax-q denominator r comes FREE from the exp's accum_out. 1/r is a
  per-partition scalar in cq-layout, so S1 = E*rinv needs no broadcast
  matmuls. S1^T is produced by a DMA xbar transpose (14ns/16x128-tile),
  eliminating the entire second score computation of v1.
- Softmax-c side: t = S2^T Xc via ecq-lhsT matmuls against natural-layout
  context tiles (ones column gives z). exp(s_c)/exp(s_q) factors cancel in
  the respective normalizations, so full scores are correct for both.
- c2q's projection goes through Y2 = Xq W2p^T (Q=O=128): W2-chunks of the
  projection collapse from 2 matmuls to 1 per chunk; c2q itself is only
  needed for the product p1 and is consumed straight from PSUM.
- Projection output DMAs straight from PSUM to DRAM (f32), no copy.
"""

import numpy as np
import ml_dtypes

import concourse.bass as bass
import concourse.mybir as mybir
from concourse import bacc
from concourse.bass import MemorySpace
from concourse.tile import TileContext
from concourse.bass_utils import run_bass_kernel_spmd

B, C, Q, E, O = 16, 2048, 128, 200, 128
NB = 2
NCORES = 8
NCT = 16          # c tiles of 128
F32 = mybir.dt.float32
F16 = mybir.dt.float16
BF = mybir.dt.bfloat16
EXP = mybir.ActivationFunctionType.Exp
MUL = mybir.AluOpType.mult
ADD = mybir.AluOpType.add

_CACHE = {}


def _build(num_devices=NCORES, reps=1):
    nc = bacc.Bacc("TRN2", target_bir_lowering=False, debug=False,
                   num_devices=num_devices)

    d_xcta = nc.dram_tensor("xcta", [NB, 128, C], BF, kind="ExternalInput")
    d_xctb = nc.dram_tensor("xctb", [NB, 73, C], BF, kind="ExternalInput")
    # natural-layout ctx tiles + ones col: [p, ct*201+j]
    d_xcn = nc.dram_tensor("xcn", [NB, 128, NCT * 201], BF,
                           kind="ExternalInput")
    d_xq = nc.dram_tensor("xq", [NB, 128, 461], BF, kind="ExternalInput")
    d_wc = nc.dram_tensor("wc", [128, 2], F32, kind="ExternalInput")
    d_wp = nc.dram_tensor("wp", [8, 128, O], BF, kind="ExternalInput")
    d_out = nc.dram_tensor("out_t", [NB, O, C], F16, kind="ExternalOutput")

    def mm(ps, lhsT, rhs, start=True, stop=True):
        nc.tensor.matmul(ps, lhsT, rhs, start=start, stop=stop)

    with TileContext(nc) as tc:
        with (
            tc.tile_pool(name="consts", bufs=1) as consts,
            tc.tile_pool(name="inputs", bufs=2) as inputs,
            tc.tile_pool(name="work", bufs=2) as work,
            tc.tile_pool(name="work1", bufs=1) as work1,
            tc.tile_pool(name="cqw", bufs=2, space=MemorySpace.PSUM) as ps_cqw,
            tc.tile_pool(name="pst", bufs=1, space=MemorySpace.PSUM) as ps_tp,
            tc.tile_pool(name="wa", bufs=4, space=MemorySpace.PSUM) as wa,
            tc.tile_pool(name="po", bufs=1, space=MemorySpace.PSUM) as po_pool,
        ):
            wp_all = consts.tile([128, 8, O], BF, tag="wp")
            wc = consts.tile([128, 2], F32, tag="wc")

            for rep in range(reps):
                # ---- all input DMAs up front (SP queue stays unblocked),
                # ordered so the first cq-group's deps land first ----
                xqs, xctas, xctbs, xcns = [], [], [], []
                for b in range(NB):
                    xq = inputs.tile([128, 461], BF, tag="xq")
                    nc.sync.dma_start(out=xq, in_=d_xq.ap()[b])
                    if rep == 0 and b == 0:
                        nc.sync.dma_start(out=wc, in_=d_wc.ap())
                    xcta = inputs.tile([128, C], BF, tag="xcta")
                    xctb = inputs.tile([73, C], BF, tag="xctb")
                    xcn = inputs.tile([128, NCT, 201], BF, tag="xcn")
                    for h in range(2):
                        csl = slice(h * 1024, (h + 1) * 1024)
                        nc.sync.dma_start(out=xcta[:, csl],
                                          in_=d_xcta.ap()[b][:, csl])
                        nc.sync.dma_start(out=xctb[:, csl],
                                          in_=d_xctb.ap()[b][:, csl])
                        if rep == 0 and b == 0 and h == 0:
                            nc.sync.dma_start(
                                out=wp_all,
                                in_=d_wp.ap().rearrange("k p o -> p k o"))
                        tsl = slice(h * 8, (h + 1) * 8)
                        nc.sync.dma_start(
                            out=xcn[:, tsl, :],
                            in_=d_xcn.ap()[b][:, h * 1608:(h + 1) * 1608])
                    xqs.append(xq)
                    xctas.append(xcta)
                    xctbs.append(xctb)
                    xcns.append(xcn)

                # ---- phase 1 (both batches): scores, softmax pieces, t ----
                # The DVE/pool tail of each batch (s1 half 1, rz, tsb) is
                # deferred into the previous batch's phase 2 so it doesn't
                # sit in front of that batch's product TTs in the queues.
                y2s, s1ts, tsbs, tails = [], [], [], []
                for b in range(NB):
                    xq, xcta, xctb, xcn = xqs[b], xctas[b], xctbs[b], xcns[b]
                    xqta = xq[:, 0:128]
                    xqtb = xq[0:73, 128:256]
                    lhsq_a = work.tile([128, 128], BF, tag="lhsq_a")
                    nc.vector.scalar_tensor_tensor(
                        lhsq_a, xqta, wc[:, 0:1],
                        xq[:, 459:460].broadcast_to([128, 128]),
                        op0=MUL, op1=ADD)
                    lhsq_b = work.tile([73, 128], BF, tag="lhsq_b")
                    nc.vector.scalar_tensor_tensor(
                        lhsq_b, xqtb, wc[0:73, 1:2],
                        xq[0:73, 460:461].broadcast_to([73, 128]),
                        op0=MUL, op1=ADD)
                    # s_q row -> psum partition 64 -> lhsq_b row 64
                    ps_sq = ps_cqw.tile([128, 512], F32, tag="cqw")
                    mm(ps_sq[64:65, 0:128], xq[:, 457:458], xqta,
                       start=True, stop=False)
                    mm(ps_sq[64:65, 0:128], xq[0:73, 458:459], xqtb,
                       start=False, stop=True)
                    nc.vector.tensor_copy(lhsq_b[64:65, :],
                                          ps_sq[64:65, 0:128])

                    ecq = work.tile([128, NCT, 128], BF, tag="ecq")
                    rcol = work.tile([128, NCT], F32, tag="rcol")
                    rinv = work.tile([128, NCT, 1], F32, tag="rinv")
                    s1 = work.tile([128, NCT, 128], BF, tag="s1")
                    s1t = work.tile([128, NCT, 128], BF, tag="s1t")
                    ps_t = ps_tp.tile([128, 512], F32, tag="pst")
                    rz = work.tile([128, 1], F32, tag="rz")
                    tsb = work.tile([128, 224], BF, tag="tsb")

                    def s1_half(h, rinv=rinv, rcol=rcol, s1=s1, ecq=ecq,
                                s1t=s1t):
                        tsl = slice(h * 8, (h + 1) * 8)
                        nc.vector.reciprocal(rinv[:, tsl, 0], rcol[:, tsl])
                        nc.gpsimd.tensor_mul(
                            s1[:, tsl, :], ecq[:, tsl, :],
                            rinv[:, tsl, :].broadcast_to([128, 8, 128]))
                        nc.sync.dma_start_transpose(
                            out=s1t[:, tsl, :], in_=s1[:, tsl, :])

                    def t_group(g, ecq=ecq, xcn=xcn, ps_t=ps_t):
                        for ct in range(4 * g, 4 * g + 4):
                            mm(ps_t[:, 0:201], ecq[:, ct, :], xcn[:, ct, :],
                               start=(ct == 0), stop=(ct == NCT - 1))

                    for g in range(4):
                        gsl = slice(g * 4, (g + 1) * 4)
                        ps = ps_cqw.tile([128, 512], F32, tag="cqw")
                        for k in range(4):
                            ct = 4 * g + k
                            csl = slice(ct * 128, (ct + 1) * 128)
                            ksl = slice(k * 128, (k + 1) * 128)
                            mm(ps[:, ksl], xcta[:, csl], lhsq_a,
                               start=True, stop=False)
                            mm(ps[:, ksl], xctb[:, csl], lhsq_b,
                               start=False, stop=True)
                        nc.scalar.activation(
                            out=ecq[:, gsl, :], in_=ps, func=EXP)
                        nc.vector.reduce_sum(
                            rcol[:, gsl], ecq[:, gsl, :],
                            axis=mybir.AxisListType.X)
                        if g > 0:
                            t_group(g - 1)
                        if g == 1:
                            s1_half(0)
                    t_group(3)
                    # Y2 = Xq @ W2p^T  [q, o]  (only needed in phase 2)
                    ps_y2 = ps_cqw.tile([128, 512], F32, tag="cqw")
                    mm(ps_y2[:, 0:128], xqta, wp_all[:, 2, :],
                       start=True, stop=False)
                    mm(ps_y2[:, 0:128], xqtb, wp_all[0:73, 3, :],
                       start=False, stop=True)
                    y2 = work.tile([128, 128], BF, tag="y2")
                    nc.scalar.copy(y2, ps_y2[:, 0:128])

                    def tail(ps_t=ps_t, rz=rz, tsb=tsb, s1_half=s1_half):
                        s1_half(1)
                        # t = S2^T Xc (exp(s_q) factor cancels via z col 200)
                        nc.vector.reciprocal(rz, ps_t[:, 200:201])
                        nc.vector.memset(tsb[:, 192:193], 0.0)
                        nc.vector.tensor_scalar_mul(
                            tsb[:, 0:192], ps_t[:, 0:192], rz)
                        nc.vector.tensor_scalar_mul(
                            tsb[:, 193:201], ps_t[:, 192:200], rz)

                    if b == 0:
                        tail()
                        tails.append(None)
                    else:
                        tails.append(tail)
                    y2s.append(y2)
                    s1ts.append(s1t)
                    tsbs.append(tsb)

                # ---- phase 2 (both batches): c2q, q2c, products, proj ----
                for b in range(NB):
                    xq, xcta, xctb = xqs[b], xctas[b], xctbs[b]
                    y2, s1t, tsb = y2s[b], s1ts[b], tsbs[b]
                    p1a = work1.tile([128, C], BF, tag="p1a")
                    p1b = work1.tile([73, C], BF, tag="p1b")
                    p2a = work1.tile([128, C], BF, tag="p2a")
                    p2b = work1.tile([73, C], BF, tag="p2b")
                    q2b = work1.tile([73, C], BF, tag="q2b")
                    out_sb = work1.tile([O, C], F16, tag="out_sb")

                    def emit_proj(ch):
                        csl = slice(ch * 512, (ch + 1) * 512)
                        s1t_ch = s1t[:, 4 * ch:4 * ch + 4, :]
                        pp = po_pool.tile([128, 512], F32, tag="po")
                        mm(pp, wp_all[:, 0, :], xcta[:, csl],
                           start=True, stop=False)
                        mm(pp, wp_all[0:73, 1, :], xctb[:, csl],
                           start=False, stop=False)
                        mm(pp, y2, s1t_ch, start=False, stop=False)
                        mm(pp, wp_all[:, 4, :], p1a[:, csl],
                           start=False, stop=False)
                        mm(pp, wp_all[0:73, 5, :], p1b[0:73, csl],
                           start=False, stop=False)
                        mm(pp, wp_all[:, 6, :], p2a[:, csl],
                           start=False, stop=False)
                        mm(pp, wp_all[0:73, 7, :], p2b[0:73, csl],
                           start=False, stop=True)
                        nc.scalar.copy(out_sb[:, csl], pp)
                        nc.sync.dma_start(out=d_out.ap()[b][:, csl],
                                          in_=out_sb[:, csl])

                    for ch in range(4):
                        csl = slice(ch * 512, (ch + 1) * 512)
                        s1t_ch = s1t[:, 4 * ch:4 * ch + 4, :]
                        pa1 = wa.tile([128, 512], F32, tag="wa")
                        mm(pa1, xq[:, 256:384], s1t_ch)
                        pb1 = wa.tile([128, 512], F32, tag="wa")
                        mm(pb1[0:73, :], xq[:, 384:457], s1t_ch)
                        pa2 = wa.tile([128, 512], F32, tag="wa")
                        mm(pa2, tsb[:, 0:128], s1t_ch)
                        pb2 = wa.tile([128, 512], F32, tag="wa")
                        mm(pb2[0:73, :], tsb[:, 128:201], s1t_ch)
                        # p1/p2a straight from psum on DVE; p2b via ACT
                        # copy + pool TT (pool is idle in phase 2 otherwise)
                        nc.vector.tensor_mul(p1a[:, csl], pa1, xcta[:, csl])
                        nc.vector.tensor_mul(p1b[0:73, csl], pb1[0:73, :],
                                             xctb[:, csl])
                        nc.vector.tensor_mul(p2a[:, csl], pa2, xcta[:, csl])
                        nc.scalar.copy(q2b[0:73, csl], pb2[0:73, :])
                        nc.gpsimd.tensor_mul(p2b[0:73, csl], q2b[0:73, csl],
                                             xctb[:, csl])
                        if ch > 0:
                            emit_proj(ch - 1)
                        # next batch's deferred phase-1 tail: emit late so
                        # its DVE/pool ops sit behind this batch's products
                        if ch == 1 and b + 1 < NB and tails[b + 1]:
                            tails[b + 1]()
                            tails[b + 1] = None
                    emit_proj(3)

    nc.compile()
    return nc


def _get_nc():
    if "nc" not in _CACHE:
        _CACHE["nc"] = _build()
    return _CACHE["nc"]


def _pack_rearranged(dst, src, row64=None):
    """dst rows 0:64 = src rows 0:64; row 64 = row64 (or 0); 65:73 = src 64:72."""
    dst[0:64] = src[0:64]
    if row64 is not None:
        dst[64] = row64
    dst[65:73] = src[64:72]


def kernel(x_contexts, x_questions, w_sim, w_proj, b_proj, _trace=False):
    bf16 = ml_dtypes.bfloat16
    x_contexts = np.ascontiguousarray(x_contexts, dtype=np.float32)
    x_questions = np.ascontiguousarray(x_questions, dtype=np.float32)
    w_sim = np.asarray(w_sim, dtype=np.float32)
    w_proj = np.asarray(w_proj, dtype=np.float32)
    b_proj = np.asarray(b_proj, dtype=np.float32)
    w1, w2, w3 = w_sim[0, 0:E], w_sim[0, E:2 * E], w_sim[0, 2 * E:]

    xct = x_contexts.transpose(0, 2, 1)            # [B, E, C]
    xcta = np.ascontiguousarray(xct[:, 0:128]).astype(bf16)
    xctb = np.zeros((B, 73, C), np.float32)
    for bi in range(B):
        _pack_rearranged(xctb[bi], xct[bi, 128:200], row64=1.0)
    xctb = xctb.astype(bf16)
    xcn = np.zeros((B, 128, NCT, 201), np.float32)
    xcn[:, :, :, 0:E] = x_contexts.reshape(B, NCT, 128, E).transpose(0, 2, 1, 3)
    xcn[:, :, :, E] = 1.0
    xcn = xcn.reshape(B, 128, NCT * 201).astype(bf16)

    xqt = x_questions.transpose(0, 2, 1)           # [B, E, Q]
    xq = np.zeros((B, 128, 461), np.float32)
    xq[:, :, 0:128] = xqt[:, 0:128]
    xq[:, 0:64, 128:256] = xqt[:, 128:192]
    xq[:, 65:73, 128:256] = xqt[:, 192:200]
    xq[:, :, 256:384] = x_questions[:, :, 0:128]
    xq[:, :, 384:448] = x_questions[:, :, 128:192]
    xq[:, :, 448] = 0.0
    xq[:, :, 449:457] = x_questions[:, :, 192:200]
    xq[:, 0:128, 457] = w2[0:128]
    xq[:, 0:64, 458] = w2[128:192]
    xq[:, 65:73, 458] = w2[192:200]
    xq[:, 0:128, 459] = w1[0:128]
    xq[:, 0:64, 460] = w1[128:192]
    xq[:, 65:73, 460] = w1[192:200]
    xq = xq.astype(bf16)

    wc = np.zeros((128, 2), np.float32)
    wc[0:128, 0] = w3[0:128]
    wc[0:64, 1] = w3[128:192]
    wc[65:73, 1] = w3[192:200]

    wpT = w_proj.T                                 # [800, O]
    wp = np.zeros((8, 128, O), np.float32)
    wp[0] = wpT[0:128]                             # W1 e0:128
    _pack_rearranged(wp[1], wpT[128:200], row64=None)
    wp[1, 64] = b_proj                             # bias pairs the ones row
    wp[2] = wpT[200:328]                           # W2^T e0:128 (Y2 rhs)
    _pack_rearranged(wp[3], wpT[328:400])
    wp[4] = wpT[400:528]                           # W3 e0:128
    _pack_rearranged(wp[5], wpT[528:600])
    wp[6] = wpT[600:728]                           # W4 e0:128
    _pack_rearranged(wp[7], wpT[728:800])
    wp = wp.astype(bf16)

    in_maps = []
    for c in range(NCORES):
        bs = slice(c * NB, (c + 1) * NB)
        in_maps.append({
            "xcta": np.ascontiguousarray(xcta[bs]),
            "xctb": np.ascontiguousarray(xctb[bs]),
            "xcn": np.ascontiguousarray(xcn[bs]),
            "xq": np.ascontiguousarray(xq[bs]),
            "wc": wc,
            "wp": wp,
        })

    nc = _get_nc()
    res = run_bass_kernel_spmd(nc, in_maps, core_ids=list(range(NCORES)),
                               trace=_trace)
    _CACHE["last_res"] = res

    out = np.empty((B, C, O), np.float32)
    for c in range(NCORES):
        ot = res.results[c]["out_t"]               # [NB, O, C] f32
        for b in range(NB):
            out[c * NB + b] = np.asarray(ot[b], dtype=np.float32).T
    return out
